# revision 26
# baseline (speedup 1.0000x reference)
"""PerJointHead Trainium2 Bass kernel.

Computes, for x [B,T,J,D]:
    xn = LayerNorm(x) * gamma + beta          (per (b,t,j) row over D)
    h  = gelu(xn @ fc1_w[j] + fc1_b[j])       (exact erf gelu)
    y  = h @ fc2_w[j] + fc2_b[j]              -> [B,T,J,3]

Sharding: data-parallel over B across 8 cores (4 B each -> 972 tokens per
joint per core).  Weights replicated.  gamma/beta are folded into
fc1_w/fc1_b on the host; x is host-padded to 1024 tokens per core so every
SBUF block is a full 128 partitions and each joint needs one input DMA.

Per-core device pipeline (per joint j):
  1. DMA x[:, j, :] (1024x512) into SBUF natural layout [128 tok, 8 blk, 512].
  2. bn_stats/bn_aggr per 128-token block -> mean/var per row (DVE).
  3. rstd = rsqrt(var+eps) via bit-hack + 3 Newton iterations (DVE only; the
     ACT Sqrt table would thrash with the Gelu table).
  4. xn = (x - mu) * rstd on DVE (tensor_scalar, per-partition scalars).
  5. x^T via PE transpose-mode (fp32 DMA transpose doesn't exist); PSUM->SBUF
     copy on ACT (rounds to fp32r) -> xt [128 d, 4 chunks, 1024 tok].
  6. fc1: out[h-chunk 128, tokens 486] = sum_c W1[c,hc].T @ xt[c]; fp32r
     matmuls (1 cycle/row at N>=256, vs 4 for fp32).
  7. gelu + fc1_b bias in one ACT pass per h-chunk (bias is per-partition in
     the [h, tokens] layout), output rounded to fp32r.
  8. fc2: out[3, tokens] accumulated over 8 h-chunks of h_act; + b2 (DVE).
  9. y^T [3, tok] -> [tok, 3] via PE transpose; gather all joints into
     ynat [128 tok, 8 blk, 17, 3]; contiguous DMA to DRAM at the end.

Engine budget per core (predicted): PE ~330 us (fc1 528k cyc + transposes +
fc2 132k cyc @ 2.4 GHz warm), DVE ~190 us, ACT ~240 us, DMA ~190 us -- PE
bound.  Every instruction is constructed to have at most 2 cross-engine wait
sources (codegen limit: "Too many sync wait commands").
"""

import os
import sys

if "/opt/trn_rl_repo" not in sys.path:
    sys.path.insert(0, "/opt/trn_rl_repo")

import numpy as np

# ---- problem constants (hardcoded per contract) ----
B, T, J, D = 32, 243, 17, 512
H = 2 * D                      # 1024
O = 3
NCORES = 8
BC = B // NCORES               # 4 batches per core
NTOK = BC * T                  # 972 valid tokens per joint per core
P = 128
DC = D // P                    # 4 contraction chunks
HC = H // P                    # 8 h chunks
NBLK = 8                       # padded token blocks
PTOK = NBLK * P                # 1024 padded tokens
NG = 2                         # moving-operand token groups for fc1/fc2
GT = NTOK // NG                # 486 (>=256 keeps fp32r at 1 cycle/row)
YLASTP = NTOK - (NBLK - 1) * P  # 76 valid tokens in the last output block
EPS = 1e-5
RSQRT_MAGIC_P1 = 0x5F3759E0    # 0x5F3759DF + 1 (magic - x == ~x + magic + 1)

_CACHE: dict = {}


def _build_module(mm_dtype_name: str = "float32r", repeat: int = 1):
    import concourse.bass as bass
    import concourse.bacc as bacc
    import concourse.tile as tile
    from concourse import mybir
    from concourse.bass import ds
    from concourse.masks import make_identity
    from contextlib import ExitStack

    f32 = mybir.dt.float32
    i32 = mybir.dt.int32
    mmdt = getattr(mybir.dt, mm_dtype_name)
    AF = mybir.ActivationFunctionType
    ALU = mybir.AluOpType

    nc = bacc.Bacc("TRN2", target_bir_lowering=False, debug=False,
                   num_devices=NCORES, enable_asserts=False)

    x_d = nc.dram_tensor("x", [PTOK, J, D], f32, kind="ExternalInput").ap()
    w1_d = nc.dram_tensor("w1", [J, P, DC, H], mmdt, kind="ExternalInput").ap()
    b1_d = nc.dram_tensor("b1", [P, J, HC], f32, kind="ExternalInput").ap()
    w2_d = nc.dram_tensor("w2", [P, J, HC, O], mmdt, kind="ExternalInput").ap()
    b2_d = nc.dram_tensor("b2", [O, J], f32, kind="ExternalInput").ap()
    y_d = nc.dram_tensor("y", [NTOK, J, O], f32, kind="ExternalOutput").ap()

    with tile.TileContext(nc) as tc, ExitStack() as ctx:
        singles = ctx.enter_context(tc.tile_pool(name="singles", bufs=1))
        xpool = ctx.enter_context(tc.tile_pool(name="xpool", bufs=2))
        wpool = ctx.enter_context(tc.tile_pool(name="wpool", bufs=2))
        xtpool = ctx.enter_context(tc.tile_pool(name="xtpool", bufs=2))
        hpool = ctx.enter_context(tc.tile_pool(name="hpool", bufs=2))
        spool = ctx.enter_context(tc.tile_pool(name="spool", bufs=2))
        ypool = ctx.enter_context(tc.tile_pool(name="ypool", bufs=2))
        psA = ctx.enter_context(tc.tile_pool(name="psA", bufs=2, space="PSUM"))
        psB = ctx.enter_context(tc.tile_pool(name="psB", bufs=4, space="PSUM"))
        psC = ctx.enter_context(tc.tile_pool(name="psC", bufs=2, space="PSUM"))

        ident_f32 = singles.tile([P, P], f32)
        make_identity(nc, ident_f32)
        ident = singles.tile([P, P], mmdt)
        nc.vector.tensor_copy(out=ident, in_=ident_f32)
        ident3 = singles.tile([O, O], f32)
        make_identity(nc, ident3)
        w2_sb = singles.tile([P, J, HC, O], mmdt)
        b1_sb = singles.tile([P, J, HC], f32)
        b2_sb = singles.tile([O, J], f32)
        # whole-core output staging: [128 tok, blk, joint, 3]
        ynat = singles.tile([P, NBLK, J, O], f32)

        rep_ctx = tc.For_i(0, repeat, 1) if repeat > 1 else None
        if rep_ctx is not None:
            ctx.enter_context(rep_ctx)

        def emit_yt(jj, ytj_t):
            # y^T [3, tok] -> [tok, 3]: all 8 block-transposes into ONE psum
            # bank, then 2 bulk copies (avoids a PE<->DVE slot-release ladder)
            psyt = psC.tile([P, NBLK, O], f32, tag="psy")
            for b in range(NBLK):
                pb = P if b < NBLK - 1 else YLASTP
                nc.tensor.transpose(psyt[:pb, b, :],
                                    ytj_t[:, ds(b * P, pb)],
                                    ident3)
            nc.vector.tensor_copy(out=ynat[:, : NBLK - 1, jj, :],
                                  in_=psyt[:, : NBLK - 1, :])
            nc.vector.tensor_copy(out=ynat[:YLASTP, NBLK - 1, jj, :],
                                  in_=psyt[:YLASTP, NBLK - 1, :])

        def emit_ydma(j0, j1):
            nc.sync.dma_start(
                out=y_d[: (NBLK - 1) * P, j0:j1, :].rearrange(
                    "(b p) j o -> p b j o", p=P),
                in_=ynat[:, : NBLK - 1, j0:j1, :],
            )
            nc.sync.dma_start(out=y_d[(NBLK - 1) * P:, j0:j1, :],
                              in_=ynat[:YLASTP, NBLK - 1, j0:j1, :])

        def prep(j):
            """Emit DMA + LayerNorm chain for joint j -> (w1_sb, xn, xt)."""
            xnat = xpool.tile([P, NBLK, D], f32, tag="xnat")
            x_dj = x_d[:, j, :].rearrange("(b p) d -> p b d", p=P)
            if j == 0:
                for lo, hi in ((0, 1), (1, 2), (2, 5), (5, 8)):
                    nc.sync.dma_start(out=xnat[:, lo:hi, :],
                                      in_=x_dj[:, lo:hi, :])
            else:
                nc.sync.dma_start(out=xnat, in_=x_dj)

            w1_sb = wpool.tile([P, DC, H], mmdt, tag="w1")
            nc.sync.dma_start(out=w1_sb, in_=w1_d[j])
            if j == 0:
                nc.sync.dma_start(out=b1_sb, in_=b1_d)
                nc.sync.dma_start(out=w2_sb, in_=w2_d)
                nc.sync.dma_start(out=b2_sb, in_=b2_d)

            stats = spool.tile([P, NBLK, 6], f32, tag="stats")
            mv = spool.tile([P, NBLK, 2], f32, tag="mv")
            vv = spool.tile([P, NBLK], f32, tag="vv")
            yi = spool.tile([P, NBLK], i32, tag="yi")
            t0 = spool.tile([P, NBLK], f32, tag="t0")
            xn = xpool.tile([P, NBLK, D], mmdt, tag="xn")
            rstd = yi.bitcast(f32)

            def emit_ln(b0, b1):
                # stats -> rstd = rsqrt(var+eps) (bit-hack + 3 Newton, DVE
                # only; the ACT Sqrt table would thrash with Gelu's) ->
                # normalize, all on DVE, for blocks [b0, b1)
                s = slice(b0, b1)
                for b in range(b0, b1):
                    nc.vector.bn_stats(out=stats[:, b, :], in_=xnat[:, b, :])
                    nc.vector.bn_aggr(out=mv[:, b, :], in_=stats[:, b, :])
                nc.vector.tensor_scalar_add(vv[:, s], mv[:, s, 1], EPS)
                nc.vector.tensor_scalar(
                    out=yi[:, s], in0=vv[:, s].bitcast(i32), scalar1=1,
                    scalar2=-1,
                    op0=ALU.logical_shift_right, op1=ALU.bitwise_xor)
                nc.vector.tensor_scalar_add(yi[:, s], yi[:, s],
                                            RSQRT_MAGIC_P1)
                for _ in range(3):
                    nc.vector.tensor_mul(t0[:, s], rstd[:, s], rstd[:, s])
                    nc.vector.tensor_mul(t0[:, s], t0[:, s], vv[:, s])
                    nc.vector.tensor_scalar(out=t0[:, s], in0=t0[:, s],
                                            scalar1=-0.5, scalar2=1.5,
                                            op0=ALU.mult, op1=ALU.add)
                    nc.vector.tensor_mul(rstd[:, s], rstd[:, s], t0[:, s])
                for b in range(b0, b1):
                    nc.vector.tensor_scalar(
                        out=xn[:, b, :], in0=xnat[:, b, :],
                        scalar1=mv[:, b, 0:1], scalar2=rstd[:, b:b + 1],
                        op0=ALU.subtract, op1=ALU.mult)

            if j == 0:
                for lo, hi in ((0, 1), (1, 2), (2, 5), (5, 8)):
                    emit_ln(lo, hi)
            else:
                emit_ln(0, NBLK)

            xt = xtpool.tile([P, DC, PTOK], mmdt, tag="xt")
            return w1_sb, xn, xt

        def emit_xT(xn_t, xt_t, b):
            # one token-block transpose (4 d-chunks) + ACT copy to SBUF
            pb = P if b < NBLK - 1 else YLASTP
            pst = psA.tile([P, DC, P], mmdt, tag="pst")
            for c in range(DC):
                nc.tensor.transpose(pst[:, c, :pb],
                                    xn_t[:pb, b, ds(c * P, P)],
                                    ident[:pb, :pb])
            nc.scalar.copy(out=xt_t[:, :, ds(b * P, pb)],
                           in_=pst[:, :, :pb])

        # Software-pipelined emission: joint j's matmul stream hosts ALL of
        # joint j+1's x-transposes (one per late fc1 h-chunk slot), so the PE
        # never executes a long transpose-only block (transpose-mode doesn't
        # count as PE-busy for the HAM clock gate).  Joint 0's transposes run
        # up front during startup.
        pending_yt = None
        w1_cur, xn_cur, xt_cur = prep(0)
        for b in range(NBLK):
            emit_xT(xn_cur, xt_cur, b)

        for j in range(J):
            nxt = prep(j + 1) if j + 1 < J else None

            if pending_yt is not None:
                jj_done = pending_yt[0]
                emit_yt(*pending_yt)
                pending_yt = None
                if jj_done in (3, 7, 11):
                    emit_ydma(jj_done - 3, jj_done + 1)

            ytj = ypool.tile([O, NTOK], f32, tag="ytj")
            for g in range(NG):
                hact = hpool.tile([P, HC, GT], mmdt, tag="hact")
                for hc in range(HC):
                    psh = psB.tile([P, GT], f32, tag="psh")
                    for c in range(DC):
                        nc.tensor.matmul(psh,
                                         w1_cur[:, c, ds(hc * P, P)],
                                         xt_cur[:, c, ds(g * GT, GT)],
                                         start=(c == 0), stop=(c == DC - 1))
                    nc.scalar.activation(out=hact[:, hc, :], in_=psh,
                                         func=AF.Gelu,
                                         bias=b1_sb[:, j, hc:hc + 1],
                                         scale=1.0)
                    if nxt is not None and hc >= HC - 4:
                        emit_xT(nxt[1], nxt[2], g * 4 + hc - (HC - 4))
                psy = psC.tile([O, GT], f32, tag="psy")
                for hc in range(HC):
                    nc.tensor.matmul(psy,
                                     w2_sb[:, j, hc, :],
                                     hact[:, hc, :],
                                     start=(hc == 0), stop=(hc == HC - 1))
                nc.scalar.activation(out=ytj[:, ds(g * GT, GT)], in_=psy,
                                     func=AF.Identity,
                                     bias=b2_sb[:, j:j + 1], scale=1.0)

            pending_yt = (j, ytj)
            if nxt is not None:
                w1_cur, xn_cur, xt_cur = nxt

        if pending_yt is not None:
            emit_yt(*pending_yt)
            pending_yt = None
        emit_ydma(12, J)

    nc.compile()
    return nc


def get_module(mm_dtype_name: str = "float32r", repeat: int = 1):
    key = ("nc", mm_dtype_name, repeat)
    if key not in _CACHE:
        _CACHE[key] = _build_module(mm_dtype_name, repeat)
    return _CACHE[key]


def _host_prep(ln_gamma, ln_beta, fc1_w, fc1_b, fc2_w, fc2_b):
    """Fold gamma/beta into fc1; reshape weights to device layouts."""
    ln_gamma = np.asarray(ln_gamma, np.float32)
    ln_beta = np.asarray(ln_beta, np.float32)
    fc1_w = np.asarray(fc1_w, np.float32)
    fc1_b = np.asarray(fc1_b, np.float32)
    fc2_w = np.asarray(fc2_w, np.float32)
    fc2_b = np.asarray(fc2_b, np.float32)

    w1p = ln_gamma[None, :, None] * fc1_w                      # [J, D, H]
    b1p = fc1_b + np.einsum("d,jdh->jh", ln_beta, fc1_w)       # [J, H]

    # lhsT layout per joint: [128 (d within chunk), DC, H]
    w1_dev = np.ascontiguousarray(
        w1p.reshape(J, DC, P, H).transpose(0, 2, 1, 3))        # [J,128,DC,H]
    b1_dev = np.ascontiguousarray(
        b1p.reshape(J, HC, P).transpose(2, 0, 1))              # [128,J,HC]
    w2_dev = np.ascontiguousarray(
        fc2_w.reshape(J, HC, P, O).transpose(2, 0, 1, 3))      # [128,J,HC,O]
    b2_dev = np.ascontiguousarray(fc2_b.T)                     # [O,J]
    return w1_dev, b1_dev, w2_dev, b2_dev


def kernel(x, ln_gamma, ln_beta, fc1_w, fc1_b, fc2_w, fc2_b):
    from concourse.bass_utils import run_bass_kernel_spmd

    x = np.asarray(x, np.float32)
    w1_dev, b1_dev, w2_dev, b2_dev = _host_prep(
        ln_gamma, ln_beta, fc1_w, fc1_b, fc2_w, fc2_b)

    nc = get_module(os.environ.get("PJH_MM_DTYPE", "float32r"))

    in_maps = []
    for c in range(NCORES):
        xc = np.zeros((PTOK, J, D), np.float32)
        xc[:NTOK] = x[c * BC:(c + 1) * BC].reshape(NTOK, J, D)
        in_maps.append({"x": xc, "w1": w1_dev, "b1": b1_dev,
                        "w2": w2_dev, "b2": b2_dev})

    trace = os.environ.get("PJH_TRACE", "0") == "1"
    res = run_bass_kernel_spmd(nc, in_maps, core_ids=list(range(NCORES)),
                               trace=trace)
    _CACHE["last_results"] = res

    y = np.concatenate(
        [r["y"].reshape(BC, T, J, O) for r in res.results], axis=0)
    return y


# revision 27
# speedup vs baseline: 294.1731x; 294.1731x over previous
"""PerJointHead Trainium2 Bass kernel.

Computes, for x [B,T,J,D]:
    xn = LayerNorm(x) * gamma + beta          (per (b,t,j) row over D)
    h  = gelu(xn @ fc1_w[j] + fc1_b[j])       (exact erf gelu)
    y  = h @ fc2_w[j] + fc2_b[j]              -> [B,T,J,3]

Sharding: data-parallel over B across 8 cores (4 B each -> 972 tokens per
joint per core).  Weights replicated.  gamma/beta are folded into
fc1_w/fc1_b on the host; x is host-padded to 1024 tokens per core so every
SBUF block is a full 128 partitions and each joint needs one input DMA.

Per-core device pipeline (per joint j):
  1. DMA x[:, j, :] (1024x512) into SBUF natural layout [128 tok, 8 blk, 512].
  2. bn_stats/bn_aggr per 128-token block -> mean/var per row (DVE).
  3. rstd = rsqrt(var+eps) via bit-hack + 3 Newton iterations (DVE only; the
     ACT Sqrt table would thrash with the Gelu table).
  4. xn = (x - mu) * rstd on DVE (tensor_scalar, per-partition scalars).
  5. x^T via PE transpose-mode (fp32 DMA transpose doesn't exist); PSUM->SBUF
     copy on ACT (rounds to fp32r) -> xt [128 d, 4 chunks, 1024 tok].
  6. fc1: out[h-chunk 128, tokens 486] = sum_c W1[c,hc].T @ xt[c]; fp32r
     matmuls (1 cycle/row at N>=256, vs 4 for fp32).
  7. gelu + fc1_b bias in one ACT pass per h-chunk (bias is per-partition in
     the [h, tokens] layout), output rounded to fp32r.
  8. fc2: out[3, tokens] accumulated over 8 h-chunks of h_act; + b2 (DVE).
  9. y^T [3, tok] -> [tok, 3] via PE transpose; gather all joints into
     ynat [128 tok, 8 blk, 17, 3]; contiguous DMA to DRAM at the end.

Engine budget per core (predicted): PE ~330 us (fc1 528k cyc + transposes +
fc2 132k cyc @ 2.4 GHz warm), DVE ~190 us, ACT ~240 us, DMA ~190 us -- PE
bound.  Every instruction is constructed to have at most 2 cross-engine wait
sources (codegen limit: "Too many sync wait commands").
"""

import os
import sys

if "/opt/trn_rl_repo" not in sys.path:
    sys.path.insert(0, "/opt/trn_rl_repo")

import numpy as np

# ---- problem constants (hardcoded per contract) ----
B, T, J, D = 32, 243, 17, 512
H = 2 * D                      # 1024
O = 3
NCORES = 8
BC = B // NCORES               # 4 batches per core
NTOK = BC * T                  # 972 valid tokens per joint per core
P = 128
DC = D // P                    # 4 contraction chunks
HC = H // P                    # 8 h chunks
NBLK = 8                       # padded token blocks
PTOK = NBLK * P                # 1024 padded tokens
NG = 2                         # moving-operand token groups for fc1/fc2
GT = NTOK // NG                # 486 (>=256 keeps fp32r at 1 cycle/row)
YLASTP = NTOK - (NBLK - 1) * P  # 76 valid tokens in the last output block
EPS = 1e-5
RSQRT_MAGIC_P1 = 0x5F3759E0    # 0x5F3759DF + 1 (magic - x == ~x + magic + 1)

_CACHE: dict = {}


def _build_module(mm_dtype_name: str = "float32r", repeat: int = 1):
    import concourse.bass as bass
    import concourse.bacc as bacc
    import concourse.tile as tile
    from concourse import mybir
    from concourse.bass import ds
    from concourse.masks import make_identity
    from contextlib import ExitStack

    f32 = mybir.dt.float32
    i32 = mybir.dt.int32
    mmdt = getattr(mybir.dt, mm_dtype_name)
    AF = mybir.ActivationFunctionType
    ALU = mybir.AluOpType

    nc = bacc.Bacc("TRN2", target_bir_lowering=False, debug=False,
                   num_devices=NCORES, enable_asserts=False)

    x_d = nc.dram_tensor("x", [PTOK, J, D], f32, kind="ExternalInput").ap()
    w1_d = nc.dram_tensor("w1", [J, P, DC, H], mmdt, kind="ExternalInput").ap()
    b1_d = nc.dram_tensor("b1", [P, J, HC], f32, kind="ExternalInput").ap()
    w2_d = nc.dram_tensor("w2", [P, J, HC, O], mmdt, kind="ExternalInput").ap()
    b2_d = nc.dram_tensor("b2", [O, J], f32, kind="ExternalInput").ap()
    y_d = nc.dram_tensor("y", [NTOK, J, O], f32, kind="ExternalOutput").ap()

    with tile.TileContext(nc) as tc, ExitStack() as ctx:
        singles = ctx.enter_context(tc.tile_pool(name="singles", bufs=1))
        xpool = ctx.enter_context(tc.tile_pool(name="xpool", bufs=2))
        wpool = ctx.enter_context(tc.tile_pool(name="wpool", bufs=2))
        xtpool = ctx.enter_context(tc.tile_pool(name="xtpool", bufs=2))
        hpool = ctx.enter_context(tc.tile_pool(name="hpool", bufs=2))
        spool = ctx.enter_context(tc.tile_pool(name="spool", bufs=2))
        ypool = ctx.enter_context(tc.tile_pool(name="ypool", bufs=2))
        psA = ctx.enter_context(tc.tile_pool(name="psA", bufs=2, space="PSUM"))
        psB = ctx.enter_context(tc.tile_pool(name="psB", bufs=4, space="PSUM"))
        psC = ctx.enter_context(tc.tile_pool(name="psC", bufs=2, space="PSUM"))

        ident_f32 = singles.tile([P, P], f32)
        make_identity(nc, ident_f32)
        ident = singles.tile([P, P], mmdt)
        nc.vector.tensor_copy(out=ident, in_=ident_f32)
        ident3 = singles.tile([O, O], f32)
        make_identity(nc, ident3)
        w2_sb = singles.tile([P, J, HC, O], mmdt)
        b1_sb = singles.tile([P, J, HC], f32)
        b2_sb = singles.tile([O, J], f32)
        # whole-core output staging: [128 tok, blk, joint, 3]
        ynat = singles.tile([P, NBLK, J, O], f32)

        rep_ctx = tc.For_i(0, repeat, 1) if repeat > 1 else None
        if rep_ctx is not None:
            ctx.enter_context(rep_ctx)

        def emit_yt(jj, ytj_t):
            # y^T [3, tok] -> [tok, 3]: all 8 block-transposes into ONE psum
            # bank, then 2 bulk copies (avoids a PE<->DVE slot-release ladder)
            psyt = psC.tile([P, NBLK, O], f32, tag="psy")
            for b in range(NBLK):
                pb = P if b < NBLK - 1 else YLASTP
                nc.tensor.transpose(psyt[:pb, b, :],
                                    ytj_t[:, ds(b * P, pb)],
                                    ident3)
            nc.vector.tensor_copy(out=ynat[:, : NBLK - 1, jj, :],
                                  in_=psyt[:, : NBLK - 1, :])
            nc.vector.tensor_copy(out=ynat[:YLASTP, NBLK - 1, jj, :],
                                  in_=psyt[:YLASTP, NBLK - 1, :])

        def emit_ydma(j0, j1):
            nc.sync.dma_start(
                out=y_d[: (NBLK - 1) * P, j0:j1, :].rearrange(
                    "(b p) j o -> p b j o", p=P),
                in_=ynat[:, : NBLK - 1, j0:j1, :],
            )
            nc.sync.dma_start(out=y_d[(NBLK - 1) * P:, j0:j1, :],
                              in_=ynat[:YLASTP, NBLK - 1, j0:j1, :])

        pending_yt = None
        for j in range(J):
            xnat = xpool.tile([P, NBLK, D], f32, tag="xnat")
            x_dj = x_d[:, j, :].rearrange("(b p) d -> p b d", p=P)
            if j == 0:
                for lo, hi in ((0, 1), (1, 2), (2, 5), (5, 8)):
                    nc.sync.dma_start(out=xnat[:, lo:hi, :],
                                      in_=x_dj[:, lo:hi, :])
            else:
                nc.sync.dma_start(out=xnat, in_=x_dj)

            w1_sb = wpool.tile([P, DC, H], mmdt, tag="w1")
            nc.sync.dma_start(out=w1_sb, in_=w1_d[j])
            if j == 0:
                nc.sync.dma_start(out=b1_sb, in_=b1_d)
                nc.sync.dma_start(out=w2_sb, in_=w2_d)
                nc.sync.dma_start(out=b2_sb, in_=b2_d)

            stats = spool.tile([P, NBLK, 6], f32, tag="stats")
            mv = spool.tile([P, NBLK, 2], f32, tag="mv")
            vv = spool.tile([P, NBLK], f32, tag="vv")
            yi = spool.tile([P, NBLK], i32, tag="yi")
            t0 = spool.tile([P, NBLK], f32, tag="t0")
            xn = xpool.tile([P, NBLK, D], mmdt, tag="xn")
            rstd = yi.bitcast(f32)

            def emit_ln(b0, b1):
                # stats -> rstd = rsqrt(var+eps) (bit-hack + 3 Newton, DVE
                # only; the ACT Sqrt table would thrash with Gelu's) ->
                # normalize on GPSIMD for blocks [b0, b1)
                s = slice(b0, b1)
                for b in range(b0, b1):
                    nc.vector.bn_stats(out=stats[:, b, :], in_=xnat[:, b, :])
                    nc.vector.bn_aggr(out=mv[:, b, :], in_=stats[:, b, :])
                nc.vector.tensor_scalar_add(vv[:, s], mv[:, s, 1], EPS)
                nc.vector.tensor_scalar(
                    out=yi[:, s], in0=vv[:, s].bitcast(i32), scalar1=1,
                    scalar2=-1,
                    op0=ALU.logical_shift_right, op1=ALU.bitwise_xor)
                nc.vector.tensor_scalar_add(yi[:, s], yi[:, s],
                                            RSQRT_MAGIC_P1)
                for _ in range(3):
                    nc.vector.tensor_mul(t0[:, s], rstd[:, s], rstd[:, s])
                    nc.vector.tensor_mul(t0[:, s], t0[:, s], vv[:, s])
                    nc.vector.tensor_scalar(out=t0[:, s], in0=t0[:, s],
                                            scalar1=-0.5, scalar2=1.5,
                                            op0=ALU.mult, op1=ALU.add)
                    nc.vector.tensor_mul(rstd[:, s], rstd[:, s], t0[:, s])
                for b in range(b0, b1):
                    nc.vector.tensor_scalar(
                        out=xn[:, b, :], in0=xnat[:, b, :],
                        scalar1=mv[:, b, 0:1], scalar2=rstd[:, b:b + 1],
                        op0=ALU.subtract, op1=ALU.mult)

            if j == 0:
                for lo, hi in ((0, 1), (1, 2), (2, 5), (5, 8)):
                    emit_ln(lo, hi)
            else:
                emit_ln(0, NBLK)

            # PE transpose -> xt [128 d, DC, PTOK]; PSUM->SBUF copy on ACT.
            # Blocks 0-3 are emitted up front (fc1 g0 reads tokens 0..485 =
            # blocks 0..3); blocks 4-7 are woven between fc1 g0 h-chunks so
            # the PE matmul stream stays dense (transpose-mode doesn't count
            # as PE-busy for the HAM clock gate).
            xt = xtpool.tile([P, DC, PTOK], mmdt, tag="xt")

            def emit_xT(b):
                pb = P if b < NBLK - 1 else YLASTP
                pst = psA.tile([P, DC, P], mmdt, tag="pst")
                for c in range(DC):
                    nc.tensor.transpose(pst[:, c, :pb],
                                        xn[:pb, b, ds(c * P, P)],
                                        ident[:pb, :pb])
                nc.scalar.copy(out=xt[:, :, ds(b * P, pb)],
                               in_=pst[:, :, :pb])

            for b in range(NBLK // 2):
                emit_xT(b)

            if pending_yt is not None:
                jj_done = pending_yt[0]
                emit_yt(*pending_yt)
                pending_yt = None
                if jj_done in (3, 7, 11):
                    emit_ydma(jj_done - 3, jj_done + 1)

            ytj = ypool.tile([O, NTOK], f32, tag="ytj")
            for g in range(NG):
                hact = hpool.tile([P, HC, GT], mmdt, tag="hact")
                for hc in range(HC):
                    psh = psB.tile([P, GT], f32, tag="psh")
                    for c in range(DC):
                        nc.tensor.matmul(psh,
                                         w1_sb[:, c, ds(hc * P, P)],
                                         xt[:, c, ds(g * GT, GT)],
                                         start=(c == 0), stop=(c == DC - 1))
                    nc.scalar.activation(out=hact[:, hc, :], in_=psh,
                                         func=AF.Gelu,
                                         bias=b1_sb[:, j, hc:hc + 1],
                                         scale=1.0)
                    if g == 0 and hc < NBLK - NBLK // 2:
                        emit_xT(NBLK // 2 + hc)
                psy = psC.tile([O, GT], f32, tag="psy")
                for hc in range(HC):
                    nc.tensor.matmul(psy,
                                     w2_sb[:, j, hc, :],
                                     hact[:, hc, :],
                                     start=(hc == 0), stop=(hc == HC - 1))
                nc.scalar.activation(out=ytj[:, ds(g * GT, GT)], in_=psy,
                                     func=AF.Identity,
                                     bias=b2_sb[:, j:j + 1], scale=1.0)

            pending_yt = (j, ytj)

        if pending_yt is not None:
            emit_yt(*pending_yt)
            pending_yt = None
        emit_ydma(12, J)

    nc.compile()
    return nc


def get_module(mm_dtype_name: str = "float32r", repeat: int = 1):
    key = ("nc", mm_dtype_name, repeat)
    if key not in _CACHE:
        _CACHE[key] = _build_module(mm_dtype_name, repeat)
    return _CACHE[key]


def _host_prep(ln_gamma, ln_beta, fc1_w, fc1_b, fc2_w, fc2_b):
    """Fold gamma/beta into fc1; reshape weights to device layouts."""
    ln_gamma = np.asarray(ln_gamma, np.float32)
    ln_beta = np.asarray(ln_beta, np.float32)
    fc1_w = np.asarray(fc1_w, np.float32)
    fc1_b = np.asarray(fc1_b, np.float32)
    fc2_w = np.asarray(fc2_w, np.float32)
    fc2_b = np.asarray(fc2_b, np.float32)

    w1p = ln_gamma[None, :, None] * fc1_w                      # [J, D, H]
    b1p = fc1_b + np.einsum("d,jdh->jh", ln_beta, fc1_w)       # [J, H]

    # lhsT layout per joint: [128 (d within chunk), DC, H]
    w1_dev = np.ascontiguousarray(
        w1p.reshape(J, DC, P, H).transpose(0, 2, 1, 3))        # [J,128,DC,H]
    b1_dev = np.ascontiguousarray(
        b1p.reshape(J, HC, P).transpose(2, 0, 1))              # [128,J,HC]
    w2_dev = np.ascontiguousarray(
        fc2_w.reshape(J, HC, P, O).transpose(2, 0, 1, 3))      # [128,J,HC,O]
    b2_dev = np.ascontiguousarray(fc2_b.T)                     # [O,J]
    return w1_dev, b1_dev, w2_dev, b2_dev


def kernel(x, ln_gamma, ln_beta, fc1_w, fc1_b, fc2_w, fc2_b):
    from concourse.bass_utils import run_bass_kernel_spmd

    x = np.asarray(x, np.float32)
    w1_dev, b1_dev, w2_dev, b2_dev = _host_prep(
        ln_gamma, ln_beta, fc1_w, fc1_b, fc2_w, fc2_b)

    nc = get_module(os.environ.get("PJH_MM_DTYPE", "float32r"))

    in_maps = []
    for c in range(NCORES):
        xc = np.zeros((PTOK, J, D), np.float32)
        xc[:NTOK] = x[c * BC:(c + 1) * BC].reshape(NTOK, J, D)
        in_maps.append({"x": xc, "w1": w1_dev, "b1": b1_dev,
                        "w2": w2_dev, "b2": b2_dev})

    trace = os.environ.get("PJH_TRACE", "0") == "1"
    res = run_bass_kernel_spmd(nc, in_maps, core_ids=list(range(NCORES)),
                               trace=trace)
    _CACHE["last_results"] = res

    y = np.concatenate(
        [r["y"].reshape(BC, T, J, O) for r in res.results], axis=0)
    return y
